# revision 25
# baseline (speedup 1.0000x reference)
"""Sparse-attention TRN2 kernel (fp16 compute, fp32 accumulate).

Reference computation (per batch b):
  pf = normalize(x @ W_pf.T); ns = normalize(x @ W_ns.T); v = x @ W_v.T
  G = pf @ pf.T                                (T x T cosine sims)
  M[u, y] = max_{j<5} G[u, start(y)+j]         (sliding window max, clamped)
  S_pf[x, y] = sum_i w_pf[i] * M[start(x)+i, y]  == (W_band @ M)[x, y]
  q[c, x] = sum_n w_ns[n] * ns.T[c, inxs[x, n]]  == (ns.T @ W_selT)[c, x]
  S_ns[x, y] = sum_c q[c, x] * ns.T[c, y]
  L = S_pf + S_ns + mask(radj);  attn = softmax(L, axis=-1);  out = attn @ v

Kernel computes L.T (y on partitions, x free) so softmax normalization and
the attn@v contraction need no transposes of the T x T tensors; the softmax
denominator comes from a ones-column appended to v. The topk gather is a
host-built sparse T x T selection matrix W_selT (4 nonzeros/row) contracted
on-device. Inputs arrive via one batched DMA per tensor (DMA issue costs
~620ns serial on the issuing engine), split across the Sync and GpSimd
queues; batch pairs share sliding-max / softmax instructions to cut
per-instruction overhead.
"""

import sys

sys.path.insert(0, "/opt/trn_rl_repo")

from contextlib import ExitStack

import numpy as np

import concourse.bacc as bacc
import concourse.bass as bass
import concourse.tile as tile
from concourse import mybir
from concourse._compat import with_exitstack

B, T, C = 32, 256, 128
TNEI = 2
TOPK = 4
NEIGH = 2 * TNEI + 1
N_CORES = 8
BPC = B // N_CORES  # batches per core

F32 = mybir.dt.float32
F16 = mybir.dt.float16
I8 = mybir.dt.int8

Act = mybir.ActivationFunctionType
Alu = mybir.AluOpType
Axis = mybir.AxisListType


def host_weights(W_pf, W_ns, W_v, v_pf, g_pf, v_ns, g_ns):
    """Constant (replicated) tensors, all pure layout/small-vector prep."""
    w_pf = (g_pf[0] * v_pf / np.linalg.norm(v_pf)).astype(np.float32)
    w_ns = (g_ns[0] * v_ns / np.linalg.norm(v_ns)).astype(np.float32)

    # Banded weight matrix: W_band[x, u] = w_pf[u - start(x)] on the band.
    start = np.clip(np.arange(T) - TNEI, 0, T - NEIGH)
    W_band = np.zeros((T, T), np.float32)
    for i in range(NEIGH):
        W_band[np.arange(T), start + i] = w_pf[i]

    WcatI = np.concatenate(
        [W_pf.T, W_ns.T, W_v.T, np.eye(C, dtype=np.float32)], axis=1
    )
    WbTt = W_band.T.reshape(2, 128, T).transpose(1, 0, 2).reshape(128, 2 * T)
    return dict(
        WcatI=np.ascontiguousarray(WcatI).astype(np.float16),
        WbT=np.ascontiguousarray(WbTt).astype(np.float16),
        _w_ns=w_ns,
    )


def host_shard(x, radj, inxs, w_ns, core):
    """Per-core input shard: batches [core*BPC, (core+1)*BPC).

    Every tensor is pre-arranged to the exact SBUF tile layout so each DMA
    is a flat partition-contiguous stream (strided DMAs cost many small
    packets and slow descriptor generation).
    """
    sl = slice(core * BPC, (core + 1) * BPC)
    # xT4 tile [C, bpc*T]: batch i at cols [i*T, (i+1)*T), x[i].T
    xT = x[sl].transpose(2, 0, 1).reshape(C, BPC * T)
    # radjT4 tile [128, bpc*2T]: [p, i*2T + u*T + t] = radj[i, t, u*128+p]
    rT = radj[sl].transpose(0, 2, 1)  # [bpc, tgt, t]
    radjT = rT.reshape(BPC, 2, 128, T).transpose(2, 0, 1, 3).reshape(128, -1)
    # W_selT[t, x] = sum_n w_ns[n] * [inxs[x, n] == t]  (duplicates add)
    WselT = np.zeros((BPC, T, T), np.float32)
    cols = np.arange(T)
    for i in range(BPC):
        ix = inxs[sl][i]
        for n in range(TOPK):
            np.add.at(WselT[i], (ix[:, n], cols), w_ns[n])
    # WselT4 tile [128, bpc*2T]: [p, i*2T + tc*T + x] = WselT[i, tc*128+p, x]
    WselTt = WselT.reshape(BPC, 2, 128, T).transpose(2, 0, 1, 3).reshape(128, -1)
    return dict(
        xT=np.ascontiguousarray(xT).astype(np.float16),
        radjT=np.ascontiguousarray(radjT).astype(np.float16),
        WselT=np.ascontiguousarray(WselTt).astype(np.float16),
    )


@with_exitstack
def emit_kernel(ctx: ExitStack, tc: tile.TileContext, io: dict, bpc: int = BPC):
    nc = tc.nc

    consts = ctx.enter_context(tc.tile_pool(name="consts", bufs=1))
    work = ctx.enter_context(tc.tile_pool(name="work", bufs=4))
    pair = ctx.enter_context(tc.tile_pool(name="pair", bufs=2))
    small = ctx.enter_context(tc.tile_pool(name="small", bufs=4))
    outp = ctx.enter_context(tc.tile_pool(name="outp", bufs=4))
    # PSUM pools are bank-granular: (tags x bufs) banks each. Total must be
    # <= 8 banks of 2KB: proj 3 + mov 1 + G 1 + LT 2 + num 1 = 8.
    ps_proj = ctx.enter_context(tc.tile_pool(name="ps_proj", bufs=3, space="PSUM"))
    ps_mov = ctx.enter_context(tc.tile_pool(name="ps_mov", bufs=1, space="PSUM"))
    ps_big = ctx.enter_context(tc.tile_pool(name="ps_big", bufs=1, space="PSUM"))
    ps_lt = ctx.enter_context(tc.tile_pool(name="ps_lt", bufs=1, space="PSUM"))
    ps_num = ps_lt

    # ---- whole-shard input tiles, one DMA each, split across engines ----
    WcatI = consts.tile([C, 4 * C], F16)  # [W_pf.T | W_ns.T | W_v.T | I]
    WbT = consts.tile([128, 2 * T], F16)  # [u-tile0 | u-tile1], each (128, 256)
    xT4 = consts.tile([C, bpc * T], F16)  # batch i at cols [i*T, (i+1)*T)
    radjT4 = consts.tile([128, bpc * 2 * T], I8)  # batch i at [i*2T, (i+1)*2T)
    WselT4 = consts.tile([128, bpc * 2 * T], F16)  # batch i: [t-tile0 | t-tile1]

    xs = io["xT"][:]
    ws = io["WselT"][:]
    rj = io["radjT"][:]
    hp = bpc // 2  # batches per DMA half
    nc.sync.dma_start(WcatI[:], io["WcatI"][:])
    for h in range(2):
        nc.sync.dma_start(
            xT4[:, h * hp * T : (h + 1) * hp * T],
            bass.AP(
                xs.tensor, xs.offset + h * hp * T,
                [[bpc * T, 128], [1, hp * T]],
            ),
        )
    nc.gpsimd.dma_start(WbT[:], io["WbT"][:])
    for h in range(2):
        nc.gpsimd.dma_start(
            WselT4[:, h * hp * 2 * T : (h + 1) * hp * 2 * T],
            bass.AP(
                ws.tensor, ws.offset + h * hp * 2 * T,
                [[bpc * 2 * T, 128], [1, hp * 2 * T]],
            ),
        )
    for h in range(2):
        nc.gpsimd.dma_start(
            radjT4[:, h * hp * 2 * T : (h + 1) * hp * 2 * T],
            bass.AP(
                rj.tensor, rj.offset + h * hp * 2 * T,
                [[bpc * 2 * T, 128], [1, hp * 2 * T]],
            ),
        )

    # Warm the Sqrt activation table during the input-DMA wait: the first
    # use otherwise pays a ~1.3us ACT_TABLE_LOAD on the critical path.
    warm0 = small.tile([128, 1], F32, tag="warm0")
    warm1 = small.tile([128, 1], F32, tag="warm1")
    nc.vector.memset(warm0[:], 1.0)
    nc.scalar.activation(warm1[:], warm0[:], Act.Sqrt)

    # Warm the PE HAM clock gate during the same dead time: ~3.4us of
    # sustained matmul activity lifts the PE clock from 1.2 to 2.4 GHz and
    # the front/back matmul cadence then keeps it there. Without this the
    # whole kernel runs at half PE clock (and run time varies with the
    # free-running HAM window phase).
    warm_mm = work.tile([128, 3 * C], F16, tag="warmmm")
    nc.vector.memset(warm_mm[:], 0.125)
    warm_ps = ps_mov.tile([C, T], F32, tag="mov", name="warmps")
    for _ in range(16):
        nc.tensor.matmul(
            warm_ps[:], warm_mm[:, 0:C], warm_mm[:, C : 3 * C],
            start=True, stop=True,
        )

    B_ = [dict() for _ in range(bpc)]  # per-batch tile registry

    def st_proj(i, b):
        b["pj0"] = ps_proj.tile([128, 3 * C], F32, tag="proj", name=f"pj0_{i}")
        b["pj1"] = ps_proj.tile([128, 3 * C], F32, tag="proj", name=f"pj1_{i}")
        x0 = i * T
        nc.tensor.matmul(
            b["pj0"][:], xT4[:, x0 : x0 + C], WcatI[:, 0 : 3 * C],
            start=True, stop=True,
        )
        nc.tensor.matmul(
            b["pj1"][:], xT4[:, x0 + C : x0 + 2 * C], WcatI[:, 0 : 3 * C],
            start=True, stop=True,
        )

    def st_early(i, b):
        # Release the pj PSUM ring fast: spill pf/ns to fp16 SBUF (scalar)
        # and copy v out (vector) right after the projection; all further
        # norm math reads SBUF. pb cols: [pf0 | ns0 | pf1 | ns1].
        b["pb"] = pb = work.tile([128, 4 * C], F16, tag="pb", name=f"pb{i}")
        nc.scalar.copy(pb[:, 0 : 2 * C], b["pj0"][:, 0 : 2 * C])
        nc.scalar.copy(pb[:, 2 * C : 4 * C], b["pj1"][:, 0 : 2 * C])
        pj = (b["pj0"], b["pj1"])
        b["v1"] = v1 = work.tile([128, 2 * (C + 1)], F16, tag="v1", name=f"v1_{i}")
        for t in range(2):
            nc.scalar.copy(
                v1[:, t * (C + 1) : t * (C + 1) + C], pj[t][:, 2 * C : 3 * C]
            )
        nc.vector.memset(
            bass.AP(v1.tensor, v1.offset + C, [v1.ap[0], [C + 1, 2], [1, 1]]),
            1.0,
        )

    def st_nrm(i, b):
        pb = b["pb"]
        sq = work.tile([128, 4 * C], F16, tag="sq", name=f"sq{i}")
        nc.gpsimd.tensor_tensor(sq[:], pb[:], pb[:], Alu.mult)
        b["nrm2"] = nrm2 = small.tile([128, 4], F32, tag="nrm2", name=f"nrm2_{i}")
        nc.vector.tensor_reduce(
            nrm2[:],
            bass.AP(sq.tensor, sq.offset, [sq.ap[0], [C, 4], [1, C]]),
            Axis.X,
            Alu.add,
        )

    def st_sqrt(i, b):
        nrm = small.tile([128, 4], F32, tag="nrm", name=f"nrm{i}")
        rinv = small.tile([128, 4], F32, tag="rinv", name=f"rinv{i}")
        nc.scalar.activation(nrm[:], b["nrm2"][:], Act.Sqrt)
        nc.vector.reciprocal(rinv[:], nrm[:])
        b["rinv"] = rinv

    def st_scale(i, b):
        # One DVE op: pfnsn = pb * rinv, rinv broadcast per quarter via a
        # stride-0 free dim. Layout [pf0 | ns0 | pf1 | ns1] like pb/rinv.
        pb, rinv = b["pb"], b["rinv"]
        b["pfnsn"] = pfnsn = work.tile(
            [128, 4 * C], F16, tag="pfnsn", name=f"pfnsn{i}"
        )
        nc.vector.tensor_tensor(
            bass.AP(pfnsn.tensor, pfnsn.offset, [pfnsn.ap[0], [C, 4], [1, C]]),
            bass.AP(pb.tensor, pb.offset, [pb.ap[0], [C, 4], [1, C]]),
            bass.AP(rinv.tensor, rinv.offset, [rinv.ap[0], [1, 4], [0, C]]),
            Alu.mult,
        )

    def st_transp(i, b):
        tp = ps_mov.tile([C, 4 * C], F16, tag="mov", name=f"tp{i}")
        for t in range(2):
            nc.tensor.transpose(
                tp[:, t * C : (t + 1) * C],
                b["pfnsn"][:, 2 * t * C : (2 * t + 1) * C],
                WcatI[:, 3 * C : 4 * C],
            )
            nc.tensor.transpose(
                tp[:, (2 + t) * C : (3 + t) * C],
                b["pfnsn"][:, (2 * t + 1) * C : (2 * t + 2) * C],
                WcatI[:, 3 * C : 4 * C],
            )
        b["pfnsT"] = pfnsT = work.tile([C, 4 * C], F16, tag="pfnsT", name=f"pt{i}")
        nc.vector.tensor_copy(pfnsT[:], tp[:])

    def st_gram(i, b):
        G = ps_big.tile([128, 2 * T], F32, tag="G", name=f"G{i}")
        for u in range(2):
            nc.tensor.matmul(
                G[:, u * T : (u + 1) * T],
                b["pfnsT"][:, u * C : (u + 1) * C],
                b["pfnsT"][:, 0:T],
                start=True,
                stop=True,
            )
        b["Gsb"] = Gsb = work.tile([128, 2 * T], F16, tag="Gsb", name=f"Gsb{i}")
        nc.scalar.copy(Gsb[:], G[:])

    def st_qT(i, b):
        # qT[c, x] = sum_t nsn[t, c] * W_selT[t, x], K=t split into 2 chunks.
        q_ps = ps_mov.tile([C, T], F32, tag="mov", name=f"qp{i}")
        w0 = i * 2 * T
        for tch in range(2):
            nc.tensor.matmul(
                q_ps[:],
                b["pfnsn"][:, (2 * tch + 1) * C : (2 * tch + 2) * C],
                WselT4[:, w0 + tch * T : w0 + (tch + 1) * T],
                start=(tch == 0),
                stop=(tch == 1),
            )
        b["q"] = q = work.tile([C, T], F16, tag="q", name=f"q{i}")
        nc.scalar.copy(q[:], q_ps[:])

    def st_slidemax(i, b):
        Gsb = b["Gsb"]
        b["M"] = M = work.tile([128, 2 * T], F16, tag="M", name=f"M{i}")
        m1 = work.tile([128, 2 * T], F16, tag="m1", name=f"m1_{i}")
        m2 = work.tile([128, 2 * T], F16, tag="m2", name=f"m2_{i}")

        def blk(tile_, off, cnt):
            return bass.AP(
                tile_.tensor, tile_.offset + off, [tile_.ap[0], [T, 2], [1, cnt]]
            )

        nc.vector.tensor_tensor(
            blk(m1, 0, T - 1), blk(Gsb, 0, T - 1), blk(Gsb, 1, T - 1), Alu.max
        )
        nc.vector.tensor_tensor(
            blk(m2, 0, T - 3), blk(m1, 0, T - 3), blk(m1, 2, T - 3), Alu.max
        )
        nc.vector.tensor_tensor(
            blk(M, 2, T - 4), blk(m2, 0, T - 4), blk(Gsb, 4, T - 4), Alu.max
        )
        nc.gpsimd.tensor_copy(
            bass.AP(M.tensor, M.offset, [M.ap[0], [T, 2], [T - 2, 2], [1, 2]]),
            bass.AP(M.tensor, M.offset + 2, [M.ap[0], [T, 2], [251, 2], [0, 2]]),
        )

    def st_logits(i, b, p):
        h = i - p
        LT2 = p_tiles[p]["LT2"]
        M = b["M"]
        for y in range(2):
            off = h * 2 * T + y * T
            nc.tensor.matmul(
                LT2[:, off : off + T],
                M[:, y * C : (y + 1) * C],
                WbT[:, 0:T],
                start=True,
                stop=False,
            )
            nc.tensor.matmul(
                LT2[:, off : off + T],
                M[:, T + y * C : T + (y + 1) * C],
                WbT[:, T : 2 * T],
                start=False,
                stop=False,
            )
            nc.tensor.matmul(
                LT2[:, off : off + T],
                b["pfnsT"][:, (2 + y) * C : (3 + y) * C],
                b["q"][:],
                start=False,
                stop=True,
            )

    def st_softmax(p):
        pt = p_tiles[p]
        PTe2 = pair.tile([128, 4 * T], F16, tag="PTe2", name=f"PTe2_{p}")
        nc.scalar.activation(PTe2[:], pt["LT2"][:], Act.Exp)
        pt["PT2"] = PT2 = pair.tile([128, 4 * T], F16, tag="PT2", name=f"PT2_{p}")
        nc.vector.tensor_tensor(
            PT2[:], PTe2[:], radjT4[:, p * 2 * T : p * 2 * T + 4 * T], Alu.mult
        )

    def st_out(i, b, p):
        h = i - p
        PT2, v1 = p_tiles[p]["PT2"], b["v1"]
        num = ps_num.tile([128, 2 * (C + 1)], F32, tag="num", name=f"num{i}")
        for xt in range(2):
            osl = slice(xt * (C + 1), (xt + 1) * (C + 1))
            for y in range(2):
                nc.tensor.matmul(
                    num[:, osl],
                    PT2[:, h * 2 * T + y * T + xt * C : h * 2 * T + y * T + (xt + 1) * C],
                    v1[:, y * (C + 1) : (y + 1) * (C + 1)],
                    start=(y == 0),
                    stop=(y == 1),
                )
        dinv = small.tile([128, 2], F32, tag="dinv", name=f"dv{i}")
        nc.vector.reciprocal(
            dinv[:],
            bass.AP(num.tensor, num.offset + C, [num.ap[0], [C + 1, 2], [1, 1]]),
        )
        out_sb = outp.tile([128, T], F32, tag="out_sb", name=f"ou{i}")
        nc.scalar.activation(
            out_sb[:, 0:C], num[:, 0:C], Act.Copy, scale=dinv[:, 0:1]
        )
        nc.vector.tensor_scalar(
            out_sb[:, C : 2 * C],
            num[:, C + 1 : 2 * C + 1],
            dinv[:, 1:2],
            None,
            Alu.mult,
        )
        od = io["out"][i]
        nc.sync.dma_start(od[:], out_sb[:])

    # ---- emission schedule ----
    for st in (st_proj, st_early, st_nrm, st_sqrt, st_scale):
        for i in range(bpc):
            st(i, B_[i])

    # Warm the Exp table after the last Sqrt (fake dep on b3's nrm2) so the
    # load overlaps slidemax instead of landing on pair0's softmax chain.
    warm2 = small.tile([128, 4], F32, tag="warm2")
    nc.scalar.activation(warm2[:], B_[bpc - 1]["nrm2"][:], Act.Exp)

    p_tiles = {}
    pairs = [(p, [i for i in (p, p + 1) if i < bpc]) for p in range(0, bpc, 2)]
    for p, members in pairs:
        p_tiles[p] = {}
        for i in members:
            st_transp(i, B_[i])
            st_gram(i, B_[i])
            st_qT(i, B_[i])
            st_slidemax(i, B_[i])
    for p, members in pairs:
        p_tiles[p]["LT2"] = ps_lt.tile(
            [128, 4 * T], F32, tag="LT", name=f"LT2_{p}"
        )
        for i in members:
            st_logits(i, B_[i], p)
        st_softmax(p)
        for i in members:
            st_out(i, B_[i], p)


def build_nc(num_cores: int = 1, bpc: int = BPC):
    nc = bacc.Bacc(None, target_bir_lowering=False, debug=False, num_swdge_queues=4)
    io = {
        "xT": nc.dram_tensor("xT", [C, bpc * T], F16, kind="ExternalInput"),
        "radjT": nc.dram_tensor(
            "radjT", [128, bpc * 2 * T], F16, kind="ExternalInput"
        ),
        "WselT": nc.dram_tensor(
            "WselT", [128, bpc * 2 * T], F16, kind="ExternalInput"
        ),
        "WcatI": nc.dram_tensor("WcatI", [C, 4 * C], F16, kind="ExternalInput"),
        "WbT": nc.dram_tensor("WbT", [128, 2 * T], F16, kind="ExternalInput"),
        "out": nc.dram_tensor(
            "out", [bpc, 128, 2 * C], F32, kind="ExternalOutput"
        ),
    }
    with tile.TileContext(nc, num_cores=num_cores) as tc:
        emit_kernel(tc, io, bpc=bpc)
    nc.compile()
    return nc


# ---------------------------------------------------------------------------
# Runner: full-input kernel() entry point.
# ---------------------------------------------------------------------------
import os
import time

_NC_CACHE = {}
LAST_RESULT = None


def _get_nc():
    if "nc" not in _NC_CACHE:
        _NC_CACHE["nc"] = build_nc(num_cores=N_CORES, bpc=BPC)
    return _NC_CACHE["nc"]


def _prep_in_maps(x, radj, inxs, W_pf, W_ns, W_v, v_pf, g_pf, v_ns, g_ns):
    x = np.asarray(x, np.float32)
    radj = np.asarray(radj, np.int32)
    inxs = np.asarray(inxs)
    consts = host_weights(
        np.asarray(W_pf, np.float32),
        np.asarray(W_ns, np.float32),
        np.asarray(W_v, np.float32),
        np.asarray(v_pf, np.float32),
        np.asarray(g_pf, np.float32),
        np.asarray(v_ns, np.float32),
        np.asarray(g_ns, np.float32),
    )
    w_ns = consts.pop("_w_ns")
    in_maps = []
    for core in range(N_CORES):
        m = dict(consts)
        m.update(host_shard(x, radj, inxs, w_ns, core))
        in_maps.append(m)
    return in_maps


def kernel(x, radj, inxs, W_pf, W_ns, W_v, v_pf, g_pf, v_ns, g_ns):
    global LAST_RESULT
    from concourse.bass_utils import run_bass_kernel_spmd

    in_maps = _prep_in_maps(
        x, radj, inxs, W_pf, W_ns, W_v, v_pf, g_pf, v_ns, g_ns
    )
    nc = _get_nc()
    res = run_bass_kernel_spmd(nc, in_maps, list(range(N_CORES)))
    LAST_RESULT = res
    # out dram layout [bpc, 128, 2C]: [i, p, xt*C+c] = out[i, xt*128+p, c]
    out = np.concatenate(
        [
            r["out"]
            .reshape(BPC, 128, 2, C)
            .transpose(0, 2, 1, 3)
            .reshape(BPC, T, C)
            for r in res.results
        ],
        axis=0,
    )
    return np.ascontiguousarray(out).astype(np.float32)


# revision 26
# speedup vs baseline: 1.0366x; 1.0366x over previous
"""Sparse-attention TRN2 kernel (fp16 compute, fp32 accumulate).

Reference computation (per batch b):
  pf = normalize(x @ W_pf.T); ns = normalize(x @ W_ns.T); v = x @ W_v.T
  G = pf @ pf.T                                (T x T cosine sims)
  M[u, y] = max_{j<5} G[u, start(y)+j]         (sliding window max, clamped)
  S_pf[x, y] = sum_i w_pf[i] * M[start(x)+i, y]  == (W_band @ M)[x, y]
  q[c, x] = sum_n w_ns[n] * ns.T[c, inxs[x, n]]  == (ns.T @ W_selT)[c, x]
  S_ns[x, y] = sum_c q[c, x] * ns.T[c, y]
  L = S_pf + S_ns + mask(radj);  attn = softmax(L, axis=-1);  out = attn @ v

Kernel computes L.T (y on partitions, x free) so softmax normalization and
the attn@v contraction need no transposes of the T x T tensors; the softmax
denominator comes from a ones-column appended to v. The topk gather is a
host-built sparse T x T selection matrix W_selT (4 nonzeros/row) contracted
on-device. Inputs arrive via one batched DMA per tensor (DMA issue costs
~620ns serial on the issuing engine), split across the Sync and GpSimd
queues; batch pairs share sliding-max / softmax instructions to cut
per-instruction overhead.
"""

import sys

sys.path.insert(0, "/opt/trn_rl_repo")

from contextlib import ExitStack

import numpy as np

import concourse.bacc as bacc
import concourse.bass as bass
import concourse.tile as tile
from concourse import mybir
from concourse._compat import with_exitstack

B, T, C = 32, 256, 128
TNEI = 2
TOPK = 4
NEIGH = 2 * TNEI + 1
N_CORES = 8
BPC = B // N_CORES  # batches per core

F32 = mybir.dt.float32
F16 = mybir.dt.float16
I8 = mybir.dt.int8

Act = mybir.ActivationFunctionType
Alu = mybir.AluOpType
Axis = mybir.AxisListType


def host_weights(W_pf, W_ns, W_v, v_pf, g_pf, v_ns, g_ns):
    """Constant (replicated) tensors, all pure layout/small-vector prep."""
    w_pf = (g_pf[0] * v_pf / np.linalg.norm(v_pf)).astype(np.float32)
    w_ns = (g_ns[0] * v_ns / np.linalg.norm(v_ns)).astype(np.float32)

    # Banded weight matrix: W_band[x, u] = w_pf[u - start(x)] on the band.
    start = np.clip(np.arange(T) - TNEI, 0, T - NEIGH)
    W_band = np.zeros((T, T), np.float32)
    for i in range(NEIGH):
        W_band[np.arange(T), start + i] = w_pf[i]

    WcatI = np.concatenate(
        [W_pf.T, W_ns.T, W_v.T, np.eye(C, dtype=np.float32)], axis=1
    )
    WbTt = W_band.T.reshape(2, 128, T).transpose(1, 0, 2).reshape(128, 2 * T)
    return dict(
        WcatI=np.ascontiguousarray(WcatI).astype(np.float16),
        WbT=np.ascontiguousarray(WbTt).astype(np.float16),
        _w_ns=w_ns,
    )


def host_shard(x, radj, inxs, w_ns, core):
    """Per-core input shard: batches [core*BPC, (core+1)*BPC).

    Every tensor is pre-arranged to the exact SBUF tile layout so each DMA
    is a flat partition-contiguous stream (strided DMAs cost many small
    packets and slow descriptor generation).
    """
    sl = slice(core * BPC, (core + 1) * BPC)
    # xT4 tile [C, bpc*T]: batch i at cols [i*T, (i+1)*T), x[i].T
    xT = x[sl].transpose(2, 0, 1).reshape(C, BPC * T)
    # radjT4 tile [128, bpc*2T]: [p, i*2T + u*T + t] = radj[i, t, u*128+p]
    rT = radj[sl].transpose(0, 2, 1)  # [bpc, tgt, t]
    radjT = rT.reshape(BPC, 2, 128, T).transpose(2, 0, 1, 3).reshape(128, -1)
    # W_selT[t, x] = sum_n w_ns[n] * [inxs[x, n] == t]  (duplicates add)
    WselT = np.zeros((BPC, T, T), np.float32)
    cols = np.arange(T)
    for i in range(BPC):
        ix = inxs[sl][i]
        for n in range(TOPK):
            np.add.at(WselT[i], (ix[:, n], cols), w_ns[n])
    # WselT4 tile [128, bpc*2T]: [p, i*2T + tc*T + x] = WselT[i, tc*128+p, x]
    WselTt = WselT.reshape(BPC, 2, 128, T).transpose(2, 0, 1, 3).reshape(128, -1)
    return dict(
        xT=np.ascontiguousarray(xT).astype(np.float16),
        radjT=np.ascontiguousarray(radjT).astype(np.float16),
        WselT=np.ascontiguousarray(WselTt).astype(np.float16),
    )


@with_exitstack
def emit_kernel(ctx: ExitStack, tc: tile.TileContext, io: dict, bpc: int = BPC):
    nc = tc.nc

    consts = ctx.enter_context(tc.tile_pool(name="consts", bufs=1))
    work = ctx.enter_context(tc.tile_pool(name="work", bufs=4))
    pair = ctx.enter_context(tc.tile_pool(name="pair", bufs=2))
    small = ctx.enter_context(tc.tile_pool(name="small", bufs=4))
    outp = ctx.enter_context(tc.tile_pool(name="outp", bufs=4))
    # PSUM pools are bank-granular: (tags x bufs) banks each. Total must be
    # <= 8 banks of 2KB: proj 3 + mov 1 + G 1 + LT 2 + num 1 = 8.
    ps_proj = ctx.enter_context(tc.tile_pool(name="ps_proj", bufs=3, space="PSUM"))
    ps_mov = ctx.enter_context(tc.tile_pool(name="ps_mov", bufs=1, space="PSUM"))
    ps_big = ctx.enter_context(tc.tile_pool(name="ps_big", bufs=1, space="PSUM"))
    ps_lt = ctx.enter_context(tc.tile_pool(name="ps_lt", bufs=1, space="PSUM"))
    ps_num = ps_lt

    # ---- whole-shard input tiles, one DMA each, split across engines ----
    WcatI = consts.tile([C, 4 * C], F16)  # [W_pf.T | W_ns.T | W_v.T | I]
    WbT = consts.tile([128, 2 * T], F16)  # [u-tile0 | u-tile1], each (128, 256)
    xT4 = consts.tile([C, bpc * T], F16)  # batch i at cols [i*T, (i+1)*T)
    radjT4 = consts.tile([128, bpc * 2 * T], I8)  # batch i at [i*2T, (i+1)*2T)
    WselT4 = consts.tile([128, bpc * 2 * T], F16)  # batch i: [t-tile0 | t-tile1]

    xs = io["xT"][:]
    ws = io["WselT"][:]
    rj = io["radjT"][:]
    hp = bpc // 2  # batches per DMA half
    nc.sync.dma_start(WcatI[:], io["WcatI"][:])
    for h in range(2):
        nc.sync.dma_start(
            xT4[:, h * hp * T : (h + 1) * hp * T],
            bass.AP(
                xs.tensor, xs.offset + h * hp * T,
                [[bpc * T, 128], [1, hp * T]],
            ),
        )
    nc.gpsimd.dma_start(WbT[:], io["WbT"][:])
    for h in range(2):
        nc.gpsimd.dma_start(
            WselT4[:, h * hp * 2 * T : (h + 1) * hp * 2 * T],
            bass.AP(
                ws.tensor, ws.offset + h * hp * 2 * T,
                [[bpc * 2 * T, 128], [1, hp * 2 * T]],
            ),
        )
    for h in range(2):
        nc.gpsimd.dma_start(
            radjT4[:, h * hp * 2 * T : (h + 1) * hp * 2 * T],
            bass.AP(
                rj.tensor, rj.offset + h * hp * 2 * T,
                [[bpc * 2 * T, 128], [1, hp * 2 * T]],
            ),
        )

    # Warm the Sqrt activation table during the input-DMA wait: the first
    # use otherwise pays a ~1.3us ACT_TABLE_LOAD on the critical path.
    warm0 = small.tile([128, 1], F32, tag="warm0")
    warm1 = small.tile([128, 1], F32, tag="warm1")
    nc.vector.memset(warm0[:], 1.0)
    nc.scalar.activation(warm1[:], warm0[:], Act.Sqrt)

    # Warm the PE HAM clock gate during the same dead time: ~3.4us of
    # sustained matmul activity lifts the PE clock from 1.2 to 2.4 GHz and
    # the front/back matmul cadence then keeps it there. Without this the
    # whole kernel runs at half PE clock (and run time varies with the
    # free-running HAM window phase).
    warm_mm = work.tile([128, 3 * C], F16, tag="warmmm")
    nc.vector.memset(warm_mm[:], 0.125)
    warm_ps = ps_mov.tile([C, T], F32, tag="mov", name="warmps")
    for _ in range(16):
        nc.tensor.matmul(
            warm_ps[:], warm_mm[:, 0:C], warm_mm[:, C : 3 * C],
            start=True, stop=True,
        )

    B_ = [dict() for _ in range(bpc)]  # per-batch tile registry

    def st_proj(i, b):
        b["pj0"] = ps_proj.tile([128, 3 * C], F32, tag="proj", name=f"pj0_{i}")
        b["pj1"] = ps_proj.tile([128, 3 * C], F32, tag="proj", name=f"pj1_{i}")
        x0 = i * T
        nc.tensor.matmul(
            b["pj0"][:], xT4[:, x0 : x0 + C], WcatI[:, 0 : 3 * C],
            start=True, stop=True,
        )
        nc.tensor.matmul(
            b["pj1"][:], xT4[:, x0 + C : x0 + 2 * C], WcatI[:, 0 : 3 * C],
            start=True, stop=True,
        )

    def st_early(i, b):
        # Release the pj PSUM ring fast: spill pf/ns to fp16 SBUF (scalar)
        # and copy v out (vector) right after the projection; all further
        # norm math reads SBUF. pb cols: [pf0 | ns0 | pf1 | ns1].
        b["pb"] = pb = work.tile([128, 4 * C], F16, tag="pb", name=f"pb{i}")
        nc.scalar.copy(pb[:, 0 : 2 * C], b["pj0"][:, 0 : 2 * C])
        nc.scalar.copy(pb[:, 2 * C : 4 * C], b["pj1"][:, 0 : 2 * C])
        pj = (b["pj0"], b["pj1"])
        b["v1"] = v1 = work.tile([128, 2 * (C + 1)], F16, tag="v1", name=f"v1_{i}")
        for t in range(2):
            nc.scalar.copy(
                v1[:, t * (C + 1) : t * (C + 1) + C], pj[t][:, 2 * C : 3 * C]
            )
        nc.vector.memset(
            bass.AP(v1.tensor, v1.offset + C, [v1.ap[0], [C + 1, 2], [1, 1]]),
            1.0,
        )

    def st_nrm(i, b):
        pb = b["pb"]
        sq = work.tile([128, 4 * C], F16, tag="sq", name=f"sq{i}")
        nc.vector.tensor_tensor(sq[:], pb[:], pb[:], Alu.mult)
        b["nrm2"] = nrm2 = small.tile([128, 4], F32, tag="nrm2", name=f"nrm2_{i}")
        nc.vector.tensor_reduce(
            nrm2[:],
            bass.AP(sq.tensor, sq.offset, [sq.ap[0], [C, 4], [1, C]]),
            Axis.X,
            Alu.add,
        )

    def st_sqrt(i, b):
        nrm = small.tile([128, 4], F32, tag="nrm", name=f"nrm{i}")
        rinv = small.tile([128, 4], F32, tag="rinv", name=f"rinv{i}")
        nc.scalar.activation(nrm[:], b["nrm2"][:], Act.Sqrt)
        nc.vector.reciprocal(rinv[:], nrm[:])
        b["rinv"] = rinv

    def st_scale(i, b):
        # One DVE op: pfnsn = pb * rinv, rinv broadcast per quarter via a
        # stride-0 free dim. Layout [pf0 | ns0 | pf1 | ns1] like pb/rinv.
        pb, rinv = b["pb"], b["rinv"]
        b["pfnsn"] = pfnsn = work.tile(
            [128, 4 * C], F16, tag="pfnsn", name=f"pfnsn{i}"
        )
        nc.vector.tensor_tensor(
            bass.AP(pfnsn.tensor, pfnsn.offset, [pfnsn.ap[0], [C, 4], [1, C]]),
            bass.AP(pb.tensor, pb.offset, [pb.ap[0], [C, 4], [1, C]]),
            bass.AP(rinv.tensor, rinv.offset, [rinv.ap[0], [1, 4], [0, C]]),
            Alu.mult,
        )

    def st_transp(i, b):
        tp = ps_mov.tile([C, 4 * C], F16, tag="mov", name=f"tp{i}")
        for t in range(2):
            nc.tensor.transpose(
                tp[:, t * C : (t + 1) * C],
                b["pfnsn"][:, 2 * t * C : (2 * t + 1) * C],
                WcatI[:, 3 * C : 4 * C],
            )
            nc.tensor.transpose(
                tp[:, (2 + t) * C : (3 + t) * C],
                b["pfnsn"][:, (2 * t + 1) * C : (2 * t + 2) * C],
                WcatI[:, 3 * C : 4 * C],
            )
        b["pfnsT"] = pfnsT = work.tile([C, 4 * C], F16, tag="pfnsT", name=f"pt{i}")
        nc.vector.tensor_copy(pfnsT[:], tp[:])

    def st_gram(i, b):
        G = ps_big.tile([128, 2 * T], F32, tag="G", name=f"G{i}")
        for u in range(2):
            nc.tensor.matmul(
                G[:, u * T : (u + 1) * T],
                b["pfnsT"][:, u * C : (u + 1) * C],
                b["pfnsT"][:, 0:T],
                start=True,
                stop=True,
            )
        b["Gsb"] = Gsb = work.tile([128, 2 * T], F16, tag="Gsb", name=f"Gsb{i}")
        nc.scalar.copy(Gsb[:], G[:])

    def st_qT(i, b):
        # qT[c, x] = sum_t nsn[t, c] * W_selT[t, x], K=t split into 2 chunks.
        q_ps = ps_mov.tile([C, T], F32, tag="mov", name=f"qp{i}")
        w0 = i * 2 * T
        for tch in range(2):
            nc.tensor.matmul(
                q_ps[:],
                b["pfnsn"][:, (2 * tch + 1) * C : (2 * tch + 2) * C],
                WselT4[:, w0 + tch * T : w0 + (tch + 1) * T],
                start=(tch == 0),
                stop=(tch == 1),
            )
        b["q"] = q = work.tile([C, T], F16, tag="q", name=f"q{i}")
        nc.scalar.copy(q[:], q_ps[:])

    def st_slidemax(i, b):
        Gsb = b["Gsb"]
        b["M"] = M = work.tile([128, 2 * T], F16, tag="M", name=f"M{i}")
        m1 = work.tile([128, 2 * T], F16, tag="m1", name=f"m1_{i}")
        m2 = work.tile([128, 2 * T], F16, tag="m2", name=f"m2_{i}")

        def blk(tile_, off, cnt):
            return bass.AP(
                tile_.tensor, tile_.offset + off, [tile_.ap[0], [T, 2], [1, cnt]]
            )

        nc.vector.tensor_tensor(
            blk(m1, 0, T - 1), blk(Gsb, 0, T - 1), blk(Gsb, 1, T - 1), Alu.max
        )
        nc.vector.tensor_tensor(
            blk(m2, 0, T - 3), blk(m1, 0, T - 3), blk(m1, 2, T - 3), Alu.max
        )
        nc.vector.tensor_tensor(
            blk(M, 2, T - 4), blk(m2, 0, T - 4), blk(Gsb, 4, T - 4), Alu.max
        )
        nc.vector.tensor_copy(
            bass.AP(M.tensor, M.offset, [M.ap[0], [T, 2], [T - 2, 2], [1, 2]]),
            bass.AP(M.tensor, M.offset + 2, [M.ap[0], [T, 2], [251, 2], [0, 2]]),
        )

    def st_logits(i, b, p):
        h = i - p
        LT2 = p_tiles[p]["LT2"]
        M = b["M"]
        for y in range(2):
            off = h * 2 * T + y * T
            nc.tensor.matmul(
                LT2[:, off : off + T],
                M[:, y * C : (y + 1) * C],
                WbT[:, 0:T],
                start=True,
                stop=False,
            )
            nc.tensor.matmul(
                LT2[:, off : off + T],
                M[:, T + y * C : T + (y + 1) * C],
                WbT[:, T : 2 * T],
                start=False,
                stop=False,
            )
            nc.tensor.matmul(
                LT2[:, off : off + T],
                b["pfnsT"][:, (2 + y) * C : (3 + y) * C],
                b["q"][:],
                start=False,
                stop=True,
            )

    def st_softmax(p):
        pt = p_tiles[p]
        PTe2 = pair.tile([128, 4 * T], F16, tag="PTe2", name=f"PTe2_{p}")
        nc.scalar.activation(PTe2[:], pt["LT2"][:], Act.Exp)
        pt["PT2"] = PT2 = pair.tile([128, 4 * T], F16, tag="PT2", name=f"PT2_{p}")
        nc.vector.tensor_tensor(
            PT2[:], PTe2[:], radjT4[:, p * 2 * T : p * 2 * T + 4 * T], Alu.mult
        )

    def st_out(i, b, p):
        h = i - p
        PT2, v1 = p_tiles[p]["PT2"], b["v1"]
        num = ps_num.tile([128, 2 * (C + 1)], F32, tag="num", name=f"num{i}")
        for xt in range(2):
            osl = slice(xt * (C + 1), (xt + 1) * (C + 1))
            for y in range(2):
                nc.tensor.matmul(
                    num[:, osl],
                    PT2[:, h * 2 * T + y * T + xt * C : h * 2 * T + y * T + (xt + 1) * C],
                    v1[:, y * (C + 1) : (y + 1) * (C + 1)],
                    start=(y == 0),
                    stop=(y == 1),
                )
        dinv = small.tile([128, 2], F32, tag="dinv", name=f"dv{i}")
        nc.vector.reciprocal(
            dinv[:],
            bass.AP(num.tensor, num.offset + C, [num.ap[0], [C + 1, 2], [1, 1]]),
        )
        out_sb = outp.tile([128, T], F32, tag="out_sb", name=f"ou{i}")
        nc.scalar.activation(
            out_sb[:, 0:C], num[:, 0:C], Act.Copy, scale=dinv[:, 0:1]
        )
        nc.vector.tensor_scalar(
            out_sb[:, C : 2 * C],
            num[:, C + 1 : 2 * C + 1],
            dinv[:, 1:2],
            None,
            Alu.mult,
        )
        od = io["out"][i]
        nc.sync.dma_start(od[:], out_sb[:])

    # ---- emission schedule ----
    for st in (st_proj, st_early, st_nrm, st_sqrt, st_scale):
        for i in range(bpc):
            st(i, B_[i])

    # Warm the Exp table after the last Sqrt (fake dep on b3's nrm2) so the
    # load overlaps slidemax instead of landing on pair0's softmax chain.
    warm2 = small.tile([128, 4], F32, tag="warm2")
    nc.scalar.activation(warm2[:], B_[bpc - 1]["nrm2"][:], Act.Exp)

    p_tiles = {}
    pairs = [(p, [i for i in (p, p + 1) if i < bpc]) for p in range(0, bpc, 2)]
    for p, members in pairs:
        p_tiles[p] = {}
        for i in members:
            st_transp(i, B_[i])
            st_gram(i, B_[i])
            st_qT(i, B_[i])
            st_slidemax(i, B_[i])
    for p, members in pairs:
        p_tiles[p]["LT2"] = ps_lt.tile(
            [128, 4 * T], F32, tag="LT", name=f"LT2_{p}"
        )
        for i in members:
            st_logits(i, B_[i], p)
        st_softmax(p)
        for i in members:
            st_out(i, B_[i], p)


def build_nc(num_cores: int = 1, bpc: int = BPC):
    nc = bacc.Bacc(None, target_bir_lowering=False, debug=False, num_swdge_queues=4)
    io = {
        "xT": nc.dram_tensor("xT", [C, bpc * T], F16, kind="ExternalInput"),
        "radjT": nc.dram_tensor(
            "radjT", [128, bpc * 2 * T], F16, kind="ExternalInput"
        ),
        "WselT": nc.dram_tensor(
            "WselT", [128, bpc * 2 * T], F16, kind="ExternalInput"
        ),
        "WcatI": nc.dram_tensor("WcatI", [C, 4 * C], F16, kind="ExternalInput"),
        "WbT": nc.dram_tensor("WbT", [128, 2 * T], F16, kind="ExternalInput"),
        "out": nc.dram_tensor(
            "out", [bpc, 128, 2 * C], F32, kind="ExternalOutput"
        ),
    }
    with tile.TileContext(nc, num_cores=num_cores) as tc:
        emit_kernel(tc, io, bpc=bpc)
    nc.compile()
    return nc


# ---------------------------------------------------------------------------
# Runner: full-input kernel() entry point.
# ---------------------------------------------------------------------------
import os
import time

_NC_CACHE = {}
LAST_RESULT = None


def _get_nc():
    if "nc" not in _NC_CACHE:
        _NC_CACHE["nc"] = build_nc(num_cores=N_CORES, bpc=BPC)
    return _NC_CACHE["nc"]


def _prep_in_maps(x, radj, inxs, W_pf, W_ns, W_v, v_pf, g_pf, v_ns, g_ns):
    x = np.asarray(x, np.float32)
    radj = np.asarray(radj, np.int32)
    inxs = np.asarray(inxs)
    consts = host_weights(
        np.asarray(W_pf, np.float32),
        np.asarray(W_ns, np.float32),
        np.asarray(W_v, np.float32),
        np.asarray(v_pf, np.float32),
        np.asarray(g_pf, np.float32),
        np.asarray(v_ns, np.float32),
        np.asarray(g_ns, np.float32),
    )
    w_ns = consts.pop("_w_ns")
    in_maps = []
    for core in range(N_CORES):
        m = dict(consts)
        m.update(host_shard(x, radj, inxs, w_ns, core))
        in_maps.append(m)
    return in_maps


def kernel(x, radj, inxs, W_pf, W_ns, W_v, v_pf, g_pf, v_ns, g_ns):
    global LAST_RESULT
    from concourse.bass_utils import run_bass_kernel_spmd

    in_maps = _prep_in_maps(
        x, radj, inxs, W_pf, W_ns, W_v, v_pf, g_pf, v_ns, g_ns
    )
    nc = _get_nc()
    res = run_bass_kernel_spmd(nc, in_maps, list(range(N_CORES)))
    LAST_RESULT = res
    # out dram layout [bpc, 128, 2C]: [i, p, xt*C+c] = out[i, xt*128+p, c]
    out = np.concatenate(
        [
            r["out"]
            .reshape(BPC, 128, 2, C)
            .transpose(0, 2, 1, 3)
            .reshape(BPC, T, C)
            for r in res.results
        ],
        axis=0,
    )
    return np.ascontiguousarray(out).astype(np.float32)
